# revision 10
# baseline (speedup 1.0000x reference)
"""Local (windowed causal) attention pathway on 8 Trainium2 NeuronCores.

Sharding: sequence parallel. Core c handles batch c//4, query rows
[(c%4)*512, (c%4)*512+512). Each core recomputes K/V for its 256-token
halo (kv range = 768 tokens, zero-padded for the first chunk), so there
are no collectives; the host concatenates the per-core outputs.

Dispatch: the axon tunnel moves ~50 MB/s, so per-call wall time is
dominated by bytes on the wire, not device compute. This version:
  - keeps the jitted shard_map executable cached across kernel() calls
    (the stock run_bass_kernel_spmd re-traces jax.jit every call);
  - keeps weights / masks / constants device-resident across calls,
    re-uploading only when the host arrays change;
  - ships x as ONE fp16 token-major array (12.6 MB incl. halo) and
    transposes/upcasts to f32 on device via PE transposes;
  - returns y fp16 token-major (8.4 MB) and upcasts on the host;
  - creates the donated output buffers on device (no zeros upload).

On-chip layout: activations are feature-major (hidden dim on SBUF
partitions, tokens on the free axis). Scores are computed transposed
(ST[kv, q] = k_raw.T @ qn) so that softmax-normalized probabilities are
directly usable as the moving operand of the PV matmul. Tricks used:
  - K-layernorm is never applied to K: since sum_d qn_d = 0, the
    (k - mk) term drops and the rstd_k scale folds into the per-
    partition `scale` operand of the exp activation.
  - The softmax denominator comes from an extra all-ones column
    appended to V (row 64 of the PV psum accumulates sum_kv P).
  - Per-token 1/l broadcast across partitions via a K=1 matmul.
"""

import os
import sys

import numpy as np

for _p in ("/opt/trn_rl_repo", os.path.expanduser("~/.axon_site/_ro/trn_rl_repo")):
    if os.path.isdir(_p) and _p not in sys.path:
        sys.path.insert(0, _p)

B, S, H = 2, 2048, 1024
NH, HD = 16, 64
WIN = 256
EPS = 1e-5

NC = 8
QLEN = 512  # queries per core
KVLEN = 768  # kv tokens per core (256 halo + 512)
PAD = 256
FT = 8  # feature tiles of 128 over H
KCH = 8  # contraction chunks of 128 over H
NJ = 6  # kv token tiles of 128
NQT = 4  # q token tiles of 128
NEG = -1.0e30

_CACHE = {}

last_results = None  # kept for test.py compatibility (always None here)


def _build_nc():
    import concourse.bass as bass
    import concourse.bacc as bacc
    import concourse.tile as tile
    from concourse import mybir
    from contextlib import ExitStack

    f32 = mybir.dt.float32
    f16 = mybir.dt.float16
    AF = mybir.ActivationFunctionType

    def r_(ap):
        # fp32r (1 cycle/row) requires producers to round to fp32r, which
        # the BIR verifier enforces; plain fp32 (4 cycles/row) is exact.
        return ap

    nc = bacc.Bacc("TRN2", target_bir_lowering=False, debug=False, num_devices=NC)

    io = {}
    # x, token-major fp16: rows 0:PAD = halo (zeros on chunk 0), PAD: = main
    io["xin"] = nc.dram_tensor("xin", [KVLEN, H], f16, kind="ExternalInput").ap()
    for w in ("wqt", "wkt", "wvt", "wot"):
        io[w] = nc.dram_tensor(w, [H, H], f32, kind="ExternalInput").ap()
    io["maskt"] = nc.dram_tensor("maskt", [NJ, 128, QLEN], f32, kind="ExternalInput").ap()
    io["eq2"] = nc.dram_tensor("eq2", [2, 128], f32, kind="ExternalInput").ap()
    io["eye2"] = nc.dram_tensor("eye2", [2, 2], f32, kind="ExternalInput").ap()
    io["eye128f"] = nc.dram_tensor("eye128f", [128, 128], f32, kind="ExternalInput").ap()
    # y, token-major fp16
    io["yt"] = nc.dram_tensor("yt", [QLEN, H], f16, kind="ExternalOutput").ap()

    with tile.TileContext(nc) as tc:
        with ExitStack() as ctx:
            ep = ctx.enter_context
            persist = ep(tc.tile_pool(name="persist", bufs=1))
            ps = ep(tc.tile_pool(name="ps", bufs=5, space="PSUM"))
            pvps = ep(tc.tile_pool(name="pvps", bufs=3, space="PSUM"))

            # ---------- constants ----------
            eq2 = persist.tile([2, 128], f32, tag="eq2")
            nc.sync.dma_start(eq2, io["eq2"])
            eye2 = persist.tile([2, 2], f32, tag="eye2")
            nc.sync.dma_start(eye2, io["eye2"])
            eye128f = persist.tile([128, 128], f32, tag="eye128f")
            nc.sync.dma_start(eye128f, io["eye128f"])
            masks = []
            for j in range(NJ):
                m = persist.tile([128, QLEN], f32, tag=f"mask{j}")
                nc.sync.dma_start(m, io["maskt"][j])
                masks.append(m)
            ones2 = persist.tile([128, 2], f32, tag="ones2")
            nc.vector.memset(ones2, 0.0)
            nc.vector.memset(ones2[0:64, 0:1], 1.0)
            nc.vector.memset(ones2[64:128, 1:2], 1.0)
            ones64 = persist.tile([65, 64], f32, tag="ones64")
            nc.vector.memset(ones64[64:65, :], 1.0)
            eps_q = persist.tile([2, 1], f32, tag="eps_q")
            nc.vector.memset(eps_q, EPS)
            eps_k = persist.tile([2, 1], f32, tag="eps_k")
            nc.vector.memset(eps_k, 64.0 * EPS)

            # persistent activations
            xts = [persist.tile([128, KVLEN], f32, tag=f"xt{c}", name=f"xt{c}") for c in range(KCH)]
            q_sb = [persist.tile([128, QLEN], f32, tag=f"q{f}", name=f"q{f}") for f in range(FT)]
            k_sb = [persist.tile([128, KVLEN], f32, tag=f"k{f}", name=f"k{f}") for f in range(FT)]
            vplus = [persist.tile([128, NH * 65], f32, tag=f"vp{t}", name=f"vp{t}") for t in range(NJ)]
            ot_sb = [persist.tile([128, QLEN], f32, tag=f"ot{f}", name=f"ot{f}") for f in range(FT)]
            rkt = [persist.tile([128, NH], f32, tag=f"rkt{j}", name=f"rkt{j}") for j in range(NJ)]
            # y, token-major fp16 tiles [128 tokens, H]
            yout = [persist.tile([128, H], f16, tag=f"yo{t}", name=f"yo{t}") for t in range(NQT)]

            # ---------- load x (fp16 token-major) and transpose to f32 feature-major ----------
            with tc.tile_pool(name="xinp", bufs=3) as xinp:
                for j in range(NJ):
                    xin_sb = xinp.tile([128, H], f16, tag="xin")
                    nc.sync.dma_start(xin_sb, io["xin"][j * 128 : (j + 1) * 128, :])
                    xin32 = xinp.tile([128, H], f32, tag="xin32")
                    nc.scalar.activation(xin32, xin_sb, AF.Copy)
                    for c in range(KCH):
                        tp = ps.tile([128, 128], f32, tag="ps")
                        nc.tensor.transpose(
                            tp, xin32[:, c * 128 : (c + 1) * 128], eye128f
                        )
                        nc.scalar.activation(
                            xts[c][:, j * 128 : (j + 1) * 128], tp, AF.Copy
                        )

            # ---------- projections ----------
            with (
                tc.tile_pool(name="wst", bufs=8) as wst,
                tc.tile_pool(name="wvst", bufs=1) as wvst,
                tc.tile_pool(name="sqp", bufs=2) as sqp,
                tc.tile_pool(name="small", bufs=6) as small,
                tc.tile_pool(name="bc", bufs=4) as bcp,
            ):
                # q projection (feature-major): q.T = Wq @ x.T over q tokens
                for f in range(FT):
                    qp = ps.tile([128, QLEN], f32, tag="ps")
                    for c in range(KCH):
                        w = wst.tile([128, 128], f32, tag="w")
                        nc.sync.dma_start(
                            w, io["wqt"][c * 128 : (c + 1) * 128, f * 128 : (f + 1) * 128]
                        )
                        nc.tensor.matmul(
                            qp,
                            r_(w),
                            r_(xts[c][:, PAD:KVLEN]),
                            start=(c == 0),
                            stop=(c == KCH - 1),
                        )
                    nc.scalar.activation(q_sb[f], qp, AF.Copy)

                # k projection (feature-major) over all kv tokens, 2 col chunks
                for f in range(FT):
                    kp1 = ps.tile([128, 512], f32, tag="ps")
                    kp2 = ps.tile([128, 256], f32, tag="ps")
                    for c in range(KCH):
                        w = wst.tile([128, 128], f32, tag="w")
                        nc.sync.dma_start(
                            w, io["wkt"][c * 128 : (c + 1) * 128, f * 128 : (f + 1) * 128]
                        )
                        nc.tensor.matmul(
                            kp1, r_(w), r_(xts[c][:, 0:512]),
                            start=(c == 0), stop=(c == KCH - 1),
                        )
                        nc.tensor.matmul(
                            kp2, r_(w), r_(xts[c][:, 512:KVLEN]),
                            start=(c == 0), stop=(c == KCH - 1),
                        )
                    nc.scalar.activation(k_sb[f][:, 0:512], kp1, AF.Copy)
                    nc.scalar.activation(k_sb[f][:, 512:KVLEN], kp2, AF.Copy)

                # v projection (token-major): v = x @ Wv.T per kv token tile
                wv_sb = []
                for c in range(KCH):
                    wv = wvst.tile([128, H], f32, tag=f"wv{c}")
                    nc.sync.dma_start(wv, io["wvt"][c * 128 : (c + 1) * 128, :])
                    wv_sb.append(wv)
                for t in range(NJ):
                    vp1 = ps.tile([128, 512], f32, tag="ps")
                    vp2 = ps.tile([128, 512], f32, tag="ps")
                    for c in range(KCH):
                        xblk = r_(xts[c][:, t * 128 : (t + 1) * 128])
                        nc.tensor.matmul(
                            vp1, xblk, r_(wv_sb[c][:, 0:512]),
                            start=(c == 0), stop=(c == KCH - 1),
                        )
                        nc.tensor.matmul(
                            vp2, xblk, r_(wv_sb[c][:, 512:H]),
                            start=(c == 0), stop=(c == KCH - 1),
                        )
                    v3 = vplus[t][:, 0 : NH * 65].rearrange("p (h d) -> p h d", d=65)
                    nc.scalar.activation(
                        v3[:, 0:8, 0:64],
                        vp1.rearrange("p (h d) -> p h d", d=64),
                        AF.Copy,
                    )
                    nc.scalar.activation(
                        v3[:, 8:16, 0:64],
                        vp2.rearrange("p (h d) -> p h d", d=64),
                        AF.Copy,
                    )
                    nc.vector.memset(v3[:, :, 64:65], 1.0)

                # ---------- q layernorm stats + apply, per feature tile ----------
                for f in range(FT):
                    sq = sqp.tile([128, QLEN], f32, tag="sq")
                    nc.vector.tensor_mul(sq, q_sb[f], q_sb[f])
                    st_sum = ps.tile([2, QLEN], f32, tag="ps")
                    nc.tensor.matmul(st_sum, r_(ones2), r_(q_sb[f]),
                                     start=True, stop=True)
                    st_sq = ps.tile([2, QLEN], f32, tag="ps")
                    nc.tensor.matmul(st_sq, r_(ones2), r_(sq),
                                     start=True, stop=True)
                    mean = small.tile([2, QLEN], f32, tag="small")
                    nc.scalar.activation(mean, st_sum, AF.Copy, scale=1.0 / 64.0)
                    msq = small.tile([2, QLEN], f32, tag="small")
                    nc.vector.tensor_mul(msq, mean, mean)
                    var = small.tile([2, QLEN], f32, tag="small")
                    nc.scalar.activation(var, st_sq, AF.Copy, scale=1.0 / 64.0)
                    nc.vector.tensor_sub(var, var, msq)
                    sd = small.tile([2, QLEN], f32, tag="small")
                    nc.scalar.activation(sd, var, AF.Sqrt, bias=eps_q)
                    rqf = small.tile([2, QLEN], f32, tag="small")
                    nc.vector.reciprocal(rqf, sd)
                    mrf = small.tile([2, QLEN], f32, tag="small")
                    nc.vector.tensor_mul(mrf, mean, rqf)
                    # broadcast across each head's 64 partitions (g folded in eq2)
                    rgp = ps.tile([128, QLEN], f32, tag="ps")
                    nc.tensor.matmul(rgp, r_(eq2), r_(rqf), start=True, stop=True)
                    mrp = ps.tile([128, QLEN], f32, tag="ps")
                    nc.tensor.matmul(mrp, r_(eq2), r_(mrf), start=True, stop=True)
                    rgb = bcp.tile([128, QLEN], f32, tag="bc")
                    nc.scalar.activation(rgb, rgp, AF.Copy)
                    mrb = bcp.tile([128, QLEN], f32, tag="bc")
                    nc.scalar.activation(mrb, mrp, AF.Copy)
                    nc.vector.tensor_mul(q_sb[f], q_sb[f], rgb)
                    nc.vector.tensor_sub(q_sb[f], q_sb[f], mrb)

                # ---------- k layernorm stats (only 0.125*rstd needed) ----------
                for f in range(FT):
                    rkf = small.tile([2, KVLEN], f32, tag="rkf")
                    for lo, hi in ((0, 512), (512, KVLEN)):
                        w_ = hi - lo
                        sqk = sqp.tile([128, 512], f32, tag="sq")
                        nc.vector.tensor_mul(
                            sqk[:, 0:w_], k_sb[f][:, lo:hi], k_sb[f][:, lo:hi]
                        )
                        stk_sum = ps.tile([2, 512], f32, tag="ps")
                        nc.tensor.matmul(
                            stk_sum[:, 0:w_], r_(ones2), r_(k_sb[f][:, lo:hi]),
                            start=True, stop=True,
                        )
                        stk_sq = ps.tile([2, 512], f32, tag="ps")
                        nc.tensor.matmul(
                            stk_sq[:, 0:w_], r_(ones2), r_(sqk[:, 0:w_]),
                            start=True, stop=True,
                        )
                        meank = small.tile([2, 512], f32, tag="small")
                        nc.scalar.activation(meank[:, 0:w_], stk_sum[:, 0:w_],
                                             AF.Copy, scale=1.0 / 64.0)
                        msqk = small.tile([2, 512], f32, tag="small")
                        nc.vector.tensor_mul(msqk[:, 0:w_], meank[:, 0:w_],
                                             meank[:, 0:w_])
                        vark = small.tile([2, 512], f32, tag="small")
                        nc.scalar.activation(vark[:, 0:w_], stk_sq[:, 0:w_],
                                             AF.Copy, scale=1.0 / 64.0)
                        nc.vector.tensor_sub(vark[:, 0:w_], vark[:, 0:w_],
                                             msqk[:, 0:w_])
                        sdk = small.tile([2, 512], f32, tag="small")
                        # sqrt(64*var + 64*eps) => reciprocal = 0.125 * rstd
                        nc.scalar.activation(sdk[:, 0:w_], vark[:, 0:w_], AF.Sqrt,
                                             scale=64.0, bias=eps_k)
                        nc.vector.reciprocal(rkf[:, lo:hi], sdk[:, 0:w_])
                    # transpose [2, 128] blocks into rkt[j][:, 2f:2f+2]
                    for j in range(NJ):
                        rp = ps.tile([128, 2], f32, tag="ps")
                        nc.tensor.transpose(
                            rp, rkf[:, j * 128 : (j + 1) * 128], eye2
                        )
                        nc.vector.tensor_copy(rkt[j][:, 2 * f : 2 * f + 2], rp)

            # ---------- attention ----------
            with (
                tc.tile_pool(name="ptp", bufs=4) as ptp,
                tc.tile_pool(name="rbp", bufs=3) as rbp,
                tc.tile_pool(name="rinvp", bufs=2) as rinvp,
                tc.tile_pool(name="otmp", bufs=2) as otmpp,
                tc.tile_pool(name="wst2", bufs=8) as wst2,
                tc.tile_pool(name="yp", bufs=2) as ypool,
            ):
                for h in range(NH):
                    f, po = h // 2, (h % 2) * 64
                    otp = pvps.tile([65, QLEN], f32, tag="pv")
                    nc.vector.memset(otp, 0.0)
                    for j in range(NJ):
                        qlo = max(0, j - 2) * 128
                        qhi = (min(NQT - 1, j) + 1) * 128
                        n = qhi - qlo
                        sp = ps.tile([128, QLEN], f32, tag="ps")
                        nc.tensor.matmul(
                            sp[:, 0:n],
                            r_(k_sb[f][po : po + 64, j * 128 : (j + 1) * 128]),
                            r_(q_sb[f][po : po + 64, qlo:qhi]),
                            start=True, stop=True,
                        )
                        nc.vector.tensor_add(sp[:, 0:n], sp[:, 0:n], masks[j][:, qlo:qhi])
                        pt = ptp.tile([128, QLEN], f32, tag="pt")
                        nc.scalar.activation(
                            pt[:, 0:n], sp[:, 0:n], AF.Exp, scale=rkt[j][:, h : h + 1]
                        )
                        nc.tensor.matmul(
                            otp[:, qlo:qhi],
                            r_(vplus[j][:, h * 65 : h * 65 + 65]),
                            r_(pt[:, 0:n]),
                            start=False, stop=(j == NJ - 1),
                            skip_group_check=True,
                        )
                    rinv = rinvp.tile([65, QLEN], f32, tag="rinv")
                    nc.vector.reciprocal(rinv[64:65, :], otp[64:65, :])
                    rbps = ps.tile([64, QLEN], f32, tag="ps")
                    nc.tensor.matmul(
                        rbps, r_(ones64[64:65, :]), r_(rinv[64:65, :]), start=True, stop=True
                    )
                    rb = rbp.tile([64, QLEN], f32, tag="rb")
                    nc.vector.tensor_copy(rb, rbps)
                    if po == 0:
                        nc.vector.tensor_mul(ot_sb[f][0:64, :], otp[0:64, :], rb)
                    else:
                        tmp = otmpp.tile([64, QLEN], f32, tag="otmp")
                        nc.vector.tensor_mul(tmp, otp[0:64, :], rb)
                        nc.sync.dma_start(ot_sb[f][64:128, :], tmp)

                # ---------- output projection (transposed to token-major fp16) ----------
                for fo in range(FT):
                    yp = ps.tile([128, QLEN], f32, tag="ps")
                    for c in range(KCH):
                        w = wst2.tile([128, 128], f32, tag="w2")
                        nc.sync.dma_start(
                            w, io["wot"][c * 128 : (c + 1) * 128, fo * 128 : (fo + 1) * 128]
                        )
                        nc.tensor.matmul(
                            yp, r_(w), r_(ot_sb[c]),
                            start=(c == 0), stop=(c == KCH - 1),
                        )
                    ysb = ypool.tile([128, QLEN], f32, tag="y")
                    nc.scalar.activation(ysb, yp, AF.Copy)
                    for t in range(NQT):
                        typ = ps.tile([128, 128], f32, tag="ps")
                        nc.tensor.transpose(
                            typ, ysb[:, t * 128 : (t + 1) * 128], eye128f
                        )
                        nc.scalar.activation(
                            yout[t][:, fo * 128 : (fo + 1) * 128], typ, AF.Copy
                        )
                for t in range(NQT):
                    nc.sync.dma_start(io["yt"][t * 128 : (t + 1) * 128, :], yout[t])

    nc.compile()
    return nc


NGROUPS = 2  # pipeline core-groups: group g = cores [g*NC/NGROUPS, ...)


class _Runner:
    """Cached jitted shard_map dispatch with device-resident parameters.

    Cores are split into NGROUPS groups, each with its own mesh + jitted
    executable, dispatched back-to-back so group g+1's H2D overlaps group
    g's D2H (the axon tunnel is full-duplex)."""

    def __init__(self):
        import jax
        import jax.numpy as jnp
        from jax.sharding import Mesh, PartitionSpec, NamedSharding
        from jax.experimental.shard_map import shard_map
        from concourse import bass2jax, mybir

        self.jax = jax
        bass2jax.install_neuronx_cc_hook()

        nc = _build_nc()
        self.nc = nc

        partition_name = (
            nc.partition_id_tensor.name if nc.partition_id_tensor else None
        )
        in_names, out_names, out_avals = [], [], []
        for alloc in nc.m.functions[0].allocations:
            if not isinstance(alloc, mybir.MemoryLocationSet):
                continue
            name = alloc.memorylocations[0].name
            if alloc.kind == "ExternalInput":
                if name != partition_name:
                    in_names.append(name)
            elif alloc.kind == "ExternalOutput":
                out_names.append(name)
                shape = tuple(alloc.tensor_shape)
                dtype = mybir.dt.np(alloc.dtype)
                out_avals.append(jax.core.ShapedArray(shape, dtype))
        self.in_names = in_names
        self.out_names = out_names
        n_params = len(in_names)
        n_outs = len(out_avals)
        all_in_names = list(in_names) + out_names + (
            [partition_name] if partition_name else []
        )
        donate = tuple(range(n_params, n_params + n_outs))

        def _body(*args):
            operands = list(args)
            if partition_name is not None:
                operands.append(bass2jax.partition_id_tensor())
            outs = bass2jax._bass_exec_p.bind(
                *operands,
                out_avals=tuple(out_avals),
                in_names=tuple(all_in_names),
                out_names=tuple(out_names),
                lowering_input_output_aliases=(),
                sim_require_finite=True,
                sim_require_nnan=True,
                nc=nc,
            )
            return tuple(outs)

        devices = jax.devices()[:NC]
        assert len(devices) == NC, f"need {NC} devices, got {len(jax.devices())}"
        gsz = NC // NGROUPS
        self.gsz = gsz
        self.groups = []
        for g in range(NGROUPS):
            gdev = devices[g * gsz : (g + 1) * gsz]
            mesh = Mesh(np.asarray(gdev), ("core",))
            shard = NamedSharding(mesh, PartitionSpec("core"))
            in_specs = (PartitionSpec("core"),) * (n_params + n_outs)
            out_specs = (PartitionSpec("core"),) * n_outs
            sharded = jax.jit(
                shard_map(
                    _body, mesh=mesh, in_specs=in_specs, out_specs=out_specs,
                    check_rep=False,
                ),
                donate_argnums=donate,
                keep_unused=True,
            )
            zshapes = [(gsz * a.shape[0], *a.shape[1:]) for a in out_avals]
            zdtypes = [a.dtype for a in out_avals]
            mkzeros = jax.jit(
                (lambda zs, zd: lambda: tuple(jnp.zeros(s, d) for s, d in zip(zs, zd)))(
                    zshapes, zdtypes
                ),
                out_shardings=tuple(shard for _ in zshapes),
            )
            self.groups.append(
                {"shard": shard, "sharded": sharded, "mkzeros": mkzeros, "dev": {}}
            )
        # host copies of the currently-uploaded weights (for change detection)
        self.host_w = None

    def put(self, g, name, arr):
        grp = self.groups[g]
        d = self.jax.device_put(arr, grp["shard"])
        grp["dev"][name] = d
        return d

    def ensure_weights(self, Wq, Wk, Wv, Wo, ln_q_w):
        key = (Wq, Wk, Wv, Wo, ln_q_w)
        if self.host_w is not None and all(
            a is b or np.array_equal(a, b) for a, b in zip(self.host_w, key)
        ):
            return
        gsz = self.gsz
        rep = lambda a: np.broadcast_to(a, (gsz, *a.shape)).reshape(
            gsz * a.shape[0], *a.shape[1:]
        )
        wts = {nm: np.ascontiguousarray(w.T) for nm, w in
               (("wqt", Wq), ("wkt", Wk), ("wvt", Wv), ("wot", Wo))}
        eq2 = _build_eq(ln_q_w)
        m_first, m_mid = _build_masks()
        for g in range(NGROUPS):
            for nm, wt in wts.items():
                self.put(g, nm, rep(wt))
            self.put(g, "eq2", rep(eq2))
            if "eye2" not in self.groups[g]["dev"]:
                self.put(g, "eye2", rep(np.eye(2, dtype=np.float32)))
                self.put(g, "eye128f", rep(np.eye(128, dtype=np.float32)))
                self.put(
                    g,
                    "maskt",
                    np.concatenate(
                        [m_first if c % 4 == 0 else m_mid
                         for c in range(g * gsz, (g + 1) * gsz)],
                        axis=0,
                    ),
                )
        self.host_w = tuple(np.array(a, copy=True) for a in key)

    def run(self, xin_builder):
        """xin_builder(g) -> [gsz*KVLEN, H] fp16. Returns [NC*QLEN, H] fp16."""
        outs = []
        for g, grp in enumerate(self.groups):
            dev_x = self.jax.device_put(xin_builder(g), grp["shard"])
            zeros = grp["mkzeros"]()
            args = [dev_x if nm == "xin" else grp["dev"][nm] for nm in self.in_names]
            out_arrs = grp["sharded"](*args, *zeros)
            outs.append(out_arrs[0])
        for o in outs:
            o.copy_to_host_async()
        return np.concatenate([np.asarray(o) for o in outs], axis=0)


def _get_runner():
    if "runner" not in _CACHE:
        _CACHE["runner"] = _Runner()
    return _CACHE["runner"]


def _build_masks():
    # maskt[j, p, q]: 0 if key (local kv index j*128+p) is visible to query
    # (local index q), else NEG. Window condition is offset-invariant:
    # 0 <= q + 256 - (j*128 + p) <= 256. Chunk-0 cores additionally blank
    # keys whose global position would be negative (the zero padding).
    j = np.arange(NJ)[:, None, None]
    p = np.arange(128)[None, :, None]
    q = np.arange(QLEN)[None, None, :]
    kv = j * 128 + p
    d = q + PAD - kv
    valid = (d >= 0) & (d <= WIN)
    m_mid = np.where(valid, 0.0, NEG).astype(np.float32)
    m_first = np.where(valid & (kv >= PAD), 0.0, NEG).astype(np.float32)
    return m_first.reshape(NJ * 128, QLEN), m_mid.reshape(NJ * 128, QLEN)


def _build_eq(ln_q_w):
    e = np.zeros((2, 128), np.float32)
    p = np.arange(128)
    e[p // 64, p] = ln_q_w[p % 64]
    return e


def _numpy_ref(x, Wq, bq, Wk, bk, Wv, bv, Wo, bo, ln_q_w, ln_q_b, ln_k_w, ln_k_b):
    # General-case fallback (not used for the spec'd inputs).
    def ln(t, g, b):
        m = t.mean(-1, keepdims=True)
        v = ((t - m) ** 2).mean(-1, keepdims=True)
        return (t - m) / np.sqrt(v + EPS) * g + b

    b_, s_ = x.shape[:2]
    q = (x @ Wq.T + bq).reshape(b_, s_, NH, HD)
    k = (x @ Wk.T + bk).reshape(b_, s_, NH, HD)
    v = (x @ Wv.T + bv).reshape(b_, s_, NH, HD)
    q = ln(q, ln_q_w, ln_q_b)
    k = ln(k, ln_k_w, ln_k_b)
    out = np.empty((b_, s_, NH * HD), np.float32)
    i = np.arange(s_)[:, None]
    jj = np.arange(s_)[None, :]
    mask = (jj <= i) & (i - jj <= WIN)
    for bi in range(b_):
        sc = np.einsum("qhd,khd->hqk", q[bi], k[bi]) / np.sqrt(HD)
        sc = np.where(mask[None], sc, -np.inf)
        sc -= sc.max(-1, keepdims=True)
        p = np.exp(sc)
        p /= p.sum(-1, keepdims=True)
        out[bi] = np.einsum("hqk,khd->qhd", p, v[bi]).reshape(s_, NH * HD)
    return out @ Wo.T + bo


def kernel(**inputs):
    x = np.asarray(inputs["x"], np.float32)
    Wq = np.asarray(inputs["Wq"], np.float32)
    Wk = np.asarray(inputs["Wk"], np.float32)
    Wv = np.asarray(inputs["Wv"], np.float32)
    Wo = np.asarray(inputs["Wo"], np.float32)
    ln_q_w = np.asarray(inputs["ln_q_w"], np.float32)
    zeros_ok = all(
        not np.any(np.asarray(inputs[nm], np.float32))
        for nm in ("bq", "bk", "bv", "bo", "ln_q_b", "ln_k_b")
    )
    lnk_ok = np.allclose(np.asarray(inputs["ln_k_w"], np.float32), 1.0)
    if not (zeros_ok and lnk_ok):
        return _numpy_ref(**{k: np.asarray(v, np.float32) for k, v in inputs.items()})

    os.environ["BASS_NEVER_TRACE"] = "1"
    r = _get_runner()
    r.ensure_weights(Wq, Wk, Wv, Wo, ln_q_w)

    # xin[c]: [KVLEN, H] fp16 token-major; rows 0:PAD halo, PAD: main chunk
    gsz = r.gsz

    def xin_builder(g):
        xg = np.zeros((gsz, KVLEN, H), np.float16)
        for i, c in enumerate(range(g * gsz, (g + 1) * gsz)):
            b, ch = c // 4, c % 4
            qs = ch * QLEN
            xg[i, PAD:] = x[b, qs : qs + QLEN]
            if ch > 0:
                xg[i, :PAD] = x[b, qs - PAD : qs]
        return xg.reshape(gsz * KVLEN, H)

    yt = r.run(xin_builder)  # [NC*QLEN, H] fp16

    out = np.empty((B, S, H), np.float32)
    yt = yt.reshape(NC, QLEN, H)
    for c in range(NC):
        b, ch = c // 4, c % 4
        out[b, ch * QLEN : (ch + 1) * QLEN, :] = yt[c]
    return out


# revision 15
# speedup vs baseline: 1.3906x; 1.3906x over previous
"""Local (windowed causal) attention pathway on 8 Trainium2 NeuronCores.

Sharding: sequence parallel. Core c handles batch c//4, query rows
[(c%4)*512, (c%4)*512+512). Each core recomputes K/V for its 256-token
halo (kv range = 768 tokens, zero-padded for the first chunk), so there
are no collectives; the host concatenates the per-core outputs.

Dispatch: the axon tunnel moves ~50 MB/s, so per-call wall time is
dominated by bytes on the wire, not device compute. This version:
  - keeps the jitted shard_map executable cached across kernel() calls
    (the stock run_bass_kernel_spmd re-traces jax.jit every call);
  - keeps weights / masks / constants device-resident across calls,
    re-uploading only when the host arrays change;
  - ships x as ONE fp16 token-major array (12.6 MB incl. halo) and
    transposes/upcasts to f32 on device via PE transposes;
  - returns y fp16 token-major (8.4 MB) and upcasts on the host;
  - creates the donated output buffers on device (no zeros upload).

On-chip layout: activations are feature-major (hidden dim on SBUF
partitions, tokens on the free axis). Scores are computed transposed
(ST[kv, q] = k_raw.T @ qn) so that softmax-normalized probabilities are
directly usable as the moving operand of the PV matmul. Tricks used:
  - K-layernorm is never applied to K: since sum_d qn_d = 0, the
    (k - mk) term drops and the rstd_k scale folds into the per-
    partition `scale` operand of the exp activation.
  - The softmax denominator comes from an extra all-ones column
    appended to V (row 64 of the PV psum accumulates sum_kv P).
  - Per-token 1/l broadcast across partitions via a K=1 matmul.
"""

import os
import sys

import numpy as np

for _p in ("/opt/trn_rl_repo", os.path.expanduser("~/.axon_site/_ro/trn_rl_repo")):
    if os.path.isdir(_p) and _p not in sys.path:
        sys.path.insert(0, _p)

B, S, H = 2, 2048, 1024
NH, HD = 16, 64
WIN = 256
EPS = 1e-5

NC = 8
QLEN = 512  # queries per core
KVLEN = 768  # kv tokens per core (256 halo + 512)
PAD = 256
FT = 8  # feature tiles of 128 over H
KCH = 8  # contraction chunks of 128 over H
NJ = 6  # kv token tiles of 128
NQT = 4  # q token tiles of 128
NEG = -1.0e30

_CACHE = {}

last_results = None  # kept for test.py compatibility (always None here)


def _build_nc():
    import concourse.bass as bass
    import concourse.bacc as bacc
    import concourse.tile as tile
    from concourse import mybir
    from contextlib import ExitStack

    f32 = mybir.dt.float32
    f16 = mybir.dt.float16
    AF = mybir.ActivationFunctionType

    def r_(ap):
        # fp32r (1 cycle/row) requires producers to round to fp32r, which
        # the BIR verifier enforces; plain fp32 (4 cycles/row) is exact.
        return ap

    nc = bacc.Bacc("TRN2", target_bir_lowering=False, debug=False, num_devices=NC)

    io = {}
    # x, token-major fp16: rows 0:PAD = halo (zeros on chunk 0), PAD: = main
    io["xin"] = nc.dram_tensor("xin", [KVLEN, H], f16, kind="ExternalInput").ap()
    for w in ("wqt", "wkt", "wvt", "wot"):
        io[w] = nc.dram_tensor(w, [H, H], f32, kind="ExternalInput").ap()
    io["maskt"] = nc.dram_tensor("maskt", [NJ, 128, QLEN], f32, kind="ExternalInput").ap()
    io["eq2"] = nc.dram_tensor("eq2", [2, 128], f32, kind="ExternalInput").ap()
    io["eye2"] = nc.dram_tensor("eye2", [2, 2], f32, kind="ExternalInput").ap()
    io["eye128f"] = nc.dram_tensor("eye128f", [128, 128], f32, kind="ExternalInput").ap()
    # y, token-major fp16
    io["yt"] = nc.dram_tensor("yt", [QLEN, H], f16, kind="ExternalOutput").ap()

    with tile.TileContext(nc) as tc:
        with ExitStack() as ctx:
            ep = ctx.enter_context
            persist = ep(tc.tile_pool(name="persist", bufs=1))
            ps = ep(tc.tile_pool(name="ps", bufs=5, space="PSUM"))
            pvps = ep(tc.tile_pool(name="pvps", bufs=3, space="PSUM"))

            # ---------- constants ----------
            eq2 = persist.tile([2, 128], f32, tag="eq2")
            nc.sync.dma_start(eq2, io["eq2"])
            eye2 = persist.tile([2, 2], f32, tag="eye2")
            nc.sync.dma_start(eye2, io["eye2"])
            eye128f = persist.tile([128, 128], f32, tag="eye128f")
            nc.sync.dma_start(eye128f, io["eye128f"])
            masks = []
            for j in range(NJ):
                m = persist.tile([128, QLEN], f32, tag=f"mask{j}")
                nc.sync.dma_start(m, io["maskt"][j])
                masks.append(m)
            ones2 = persist.tile([128, 2], f32, tag="ones2")
            nc.vector.memset(ones2, 0.0)
            nc.vector.memset(ones2[0:64, 0:1], 1.0)
            nc.vector.memset(ones2[64:128, 1:2], 1.0)
            ones64 = persist.tile([65, 64], f32, tag="ones64")
            nc.vector.memset(ones64[64:65, :], 1.0)
            eps_q = persist.tile([2, 1], f32, tag="eps_q")
            nc.vector.memset(eps_q, EPS)
            eps_k = persist.tile([2, 1], f32, tag="eps_k")
            nc.vector.memset(eps_k, 64.0 * EPS)

            # persistent activations
            xts = [persist.tile([128, KVLEN], f32, tag=f"xt{c}", name=f"xt{c}") for c in range(KCH)]
            q_sb = [persist.tile([128, QLEN], f32, tag=f"q{f}", name=f"q{f}") for f in range(FT)]
            k_sb = [persist.tile([128, KVLEN], f32, tag=f"k{f}", name=f"k{f}") for f in range(FT)]
            vplus = [persist.tile([128, NH * 65], f32, tag=f"vp{t}", name=f"vp{t}") for t in range(NJ)]
            ot_sb = [persist.tile([128, QLEN], f32, tag=f"ot{f}", name=f"ot{f}") for f in range(FT)]
            rkt = [persist.tile([128, NH], f32, tag=f"rkt{j}", name=f"rkt{j}") for j in range(NJ)]
            # y, token-major fp16 tiles [128 tokens, H]
            yout = [persist.tile([128, H], f16, tag=f"yo{t}", name=f"yo{t}") for t in range(NQT)]

            # ---------- load x (fp16 token-major) and transpose to f32 feature-major ----------
            with tc.tile_pool(name="xinp", bufs=3) as xinp:
                for j in range(NJ):
                    xin_sb = xinp.tile([128, H], f16, tag="xin")
                    nc.sync.dma_start(xin_sb, io["xin"][j * 128 : (j + 1) * 128, :])
                    xin32 = xinp.tile([128, H], f32, tag="xin32")
                    nc.scalar.activation(xin32, xin_sb, AF.Copy)
                    for c in range(KCH):
                        tp = ps.tile([128, 128], f32, tag="ps")
                        nc.tensor.transpose(
                            tp, xin32[:, c * 128 : (c + 1) * 128], eye128f
                        )
                        nc.scalar.activation(
                            xts[c][:, j * 128 : (j + 1) * 128], tp, AF.Copy
                        )

            # ---------- projections ----------
            with (
                tc.tile_pool(name="wst", bufs=8) as wst,
                tc.tile_pool(name="wvst", bufs=1) as wvst,
                tc.tile_pool(name="sqp", bufs=2) as sqp,
                tc.tile_pool(name="small", bufs=6) as small,
                tc.tile_pool(name="bc", bufs=4) as bcp,
            ):
                # q projection (feature-major): q.T = Wq @ x.T over q tokens
                for f in range(FT):
                    qp = ps.tile([128, QLEN], f32, tag="ps")
                    for c in range(KCH):
                        w = wst.tile([128, 128], f32, tag="w")
                        nc.sync.dma_start(
                            w, io["wqt"][c * 128 : (c + 1) * 128, f * 128 : (f + 1) * 128]
                        )
                        nc.tensor.matmul(
                            qp,
                            r_(w),
                            r_(xts[c][:, PAD:KVLEN]),
                            start=(c == 0),
                            stop=(c == KCH - 1),
                        )
                    nc.scalar.activation(q_sb[f], qp, AF.Copy)

                # k projection (feature-major) over all kv tokens, 2 col chunks
                for f in range(FT):
                    kp1 = ps.tile([128, 512], f32, tag="ps")
                    kp2 = ps.tile([128, 256], f32, tag="ps")
                    for c in range(KCH):
                        w = wst.tile([128, 128], f32, tag="w")
                        nc.sync.dma_start(
                            w, io["wkt"][c * 128 : (c + 1) * 128, f * 128 : (f + 1) * 128]
                        )
                        nc.tensor.matmul(
                            kp1, r_(w), r_(xts[c][:, 0:512]),
                            start=(c == 0), stop=(c == KCH - 1),
                        )
                        nc.tensor.matmul(
                            kp2, r_(w), r_(xts[c][:, 512:KVLEN]),
                            start=(c == 0), stop=(c == KCH - 1),
                        )
                    nc.scalar.activation(k_sb[f][:, 0:512], kp1, AF.Copy)
                    nc.scalar.activation(k_sb[f][:, 512:KVLEN], kp2, AF.Copy)

                # v projection (token-major): v = x @ Wv.T per kv token tile
                wv_sb = []
                for c in range(KCH):
                    wv = wvst.tile([128, H], f32, tag=f"wv{c}")
                    nc.sync.dma_start(wv, io["wvt"][c * 128 : (c + 1) * 128, :])
                    wv_sb.append(wv)
                for t in range(NJ):
                    vp1 = ps.tile([128, 512], f32, tag="ps")
                    vp2 = ps.tile([128, 512], f32, tag="ps")
                    for c in range(KCH):
                        xblk = r_(xts[c][:, t * 128 : (t + 1) * 128])
                        nc.tensor.matmul(
                            vp1, xblk, r_(wv_sb[c][:, 0:512]),
                            start=(c == 0), stop=(c == KCH - 1),
                        )
                        nc.tensor.matmul(
                            vp2, xblk, r_(wv_sb[c][:, 512:H]),
                            start=(c == 0), stop=(c == KCH - 1),
                        )
                    v3 = vplus[t][:, 0 : NH * 65].rearrange("p (h d) -> p h d", d=65)
                    nc.scalar.activation(
                        v3[:, 0:8, 0:64],
                        vp1.rearrange("p (h d) -> p h d", d=64),
                        AF.Copy,
                    )
                    nc.scalar.activation(
                        v3[:, 8:16, 0:64],
                        vp2.rearrange("p (h d) -> p h d", d=64),
                        AF.Copy,
                    )
                    nc.vector.memset(v3[:, :, 64:65], 1.0)

                # ---------- q layernorm stats + apply, per feature tile ----------
                for f in range(FT):
                    sq = sqp.tile([128, QLEN], f32, tag="sq")
                    nc.vector.tensor_mul(sq, q_sb[f], q_sb[f])
                    st_sum = ps.tile([2, QLEN], f32, tag="ps")
                    nc.tensor.matmul(st_sum, r_(ones2), r_(q_sb[f]),
                                     start=True, stop=True)
                    st_sq = ps.tile([2, QLEN], f32, tag="ps")
                    nc.tensor.matmul(st_sq, r_(ones2), r_(sq),
                                     start=True, stop=True)
                    mean = small.tile([2, QLEN], f32, tag="small")
                    nc.scalar.activation(mean, st_sum, AF.Copy, scale=1.0 / 64.0)
                    msq = small.tile([2, QLEN], f32, tag="small")
                    nc.vector.tensor_mul(msq, mean, mean)
                    var = small.tile([2, QLEN], f32, tag="small")
                    nc.scalar.activation(var, st_sq, AF.Copy, scale=1.0 / 64.0)
                    nc.vector.tensor_sub(var, var, msq)
                    sd = small.tile([2, QLEN], f32, tag="small")
                    nc.scalar.activation(sd, var, AF.Sqrt, bias=eps_q)
                    rqf = small.tile([2, QLEN], f32, tag="small")
                    nc.vector.reciprocal(rqf, sd)
                    mrf = small.tile([2, QLEN], f32, tag="small")
                    nc.vector.tensor_mul(mrf, mean, rqf)
                    # broadcast across each head's 64 partitions (g folded in eq2)
                    rgp = ps.tile([128, QLEN], f32, tag="ps")
                    nc.tensor.matmul(rgp, r_(eq2), r_(rqf), start=True, stop=True)
                    mrp = ps.tile([128, QLEN], f32, tag="ps")
                    nc.tensor.matmul(mrp, r_(eq2), r_(mrf), start=True, stop=True)
                    rgb = bcp.tile([128, QLEN], f32, tag="bc")
                    nc.scalar.activation(rgb, rgp, AF.Copy)
                    mrb = bcp.tile([128, QLEN], f32, tag="bc")
                    nc.scalar.activation(mrb, mrp, AF.Copy)
                    nc.vector.tensor_mul(q_sb[f], q_sb[f], rgb)
                    nc.vector.tensor_sub(q_sb[f], q_sb[f], mrb)

                # ---------- k layernorm stats (only 0.125*rstd needed) ----------
                for f in range(FT):
                    rkf = small.tile([2, KVLEN], f32, tag="rkf")
                    for lo, hi in ((0, 512), (512, KVLEN)):
                        w_ = hi - lo
                        sqk = sqp.tile([128, 512], f32, tag="sq")
                        nc.vector.tensor_mul(
                            sqk[:, 0:w_], k_sb[f][:, lo:hi], k_sb[f][:, lo:hi]
                        )
                        stk_sum = ps.tile([2, 512], f32, tag="ps")
                        nc.tensor.matmul(
                            stk_sum[:, 0:w_], r_(ones2), r_(k_sb[f][:, lo:hi]),
                            start=True, stop=True,
                        )
                        stk_sq = ps.tile([2, 512], f32, tag="ps")
                        nc.tensor.matmul(
                            stk_sq[:, 0:w_], r_(ones2), r_(sqk[:, 0:w_]),
                            start=True, stop=True,
                        )
                        meank = small.tile([2, 512], f32, tag="small")
                        nc.scalar.activation(meank[:, 0:w_], stk_sum[:, 0:w_],
                                             AF.Copy, scale=1.0 / 64.0)
                        msqk = small.tile([2, 512], f32, tag="small")
                        nc.vector.tensor_mul(msqk[:, 0:w_], meank[:, 0:w_],
                                             meank[:, 0:w_])
                        vark = small.tile([2, 512], f32, tag="small")
                        nc.scalar.activation(vark[:, 0:w_], stk_sq[:, 0:w_],
                                             AF.Copy, scale=1.0 / 64.0)
                        nc.vector.tensor_sub(vark[:, 0:w_], vark[:, 0:w_],
                                             msqk[:, 0:w_])
                        sdk = small.tile([2, 512], f32, tag="small")
                        # sqrt(64*var + 64*eps) => reciprocal = 0.125 * rstd
                        nc.scalar.activation(sdk[:, 0:w_], vark[:, 0:w_], AF.Sqrt,
                                             scale=64.0, bias=eps_k)
                        nc.vector.reciprocal(rkf[:, lo:hi], sdk[:, 0:w_])
                    # transpose [2, 128] blocks into rkt[j][:, 2f:2f+2]
                    for j in range(NJ):
                        rp = ps.tile([128, 2], f32, tag="ps")
                        nc.tensor.transpose(
                            rp, rkf[:, j * 128 : (j + 1) * 128], eye2
                        )
                        nc.vector.tensor_copy(rkt[j][:, 2 * f : 2 * f + 2], rp)

            # ---------- attention ----------
            with (
                tc.tile_pool(name="ptp", bufs=4) as ptp,
                tc.tile_pool(name="rbp", bufs=3) as rbp,
                tc.tile_pool(name="rinvp", bufs=2) as rinvp,
                tc.tile_pool(name="otmp", bufs=2) as otmpp,
                tc.tile_pool(name="wst2", bufs=8) as wst2,
                tc.tile_pool(name="yp", bufs=2) as ypool,
            ):
                for h in range(NH):
                    f, po = h // 2, (h % 2) * 64
                    otp = pvps.tile([65, QLEN], f32, tag="pv")
                    nc.vector.memset(otp, 0.0)
                    for j in range(NJ):
                        qlo = max(0, j - 2) * 128
                        qhi = (min(NQT - 1, j) + 1) * 128
                        n = qhi - qlo
                        sp = ps.tile([128, QLEN], f32, tag="ps")
                        nc.tensor.matmul(
                            sp[:, 0:n],
                            r_(k_sb[f][po : po + 64, j * 128 : (j + 1) * 128]),
                            r_(q_sb[f][po : po + 64, qlo:qhi]),
                            start=True, stop=True,
                        )
                        nc.vector.tensor_add(sp[:, 0:n], sp[:, 0:n], masks[j][:, qlo:qhi])
                        pt = ptp.tile([128, QLEN], f32, tag="pt")
                        nc.scalar.activation(
                            pt[:, 0:n], sp[:, 0:n], AF.Exp, scale=rkt[j][:, h : h + 1]
                        )
                        nc.tensor.matmul(
                            otp[:, qlo:qhi],
                            r_(vplus[j][:, h * 65 : h * 65 + 65]),
                            r_(pt[:, 0:n]),
                            start=False, stop=(j == NJ - 1),
                            skip_group_check=True,
                        )
                    rinv = rinvp.tile([65, QLEN], f32, tag="rinv")
                    nc.vector.reciprocal(rinv[64:65, :], otp[64:65, :])
                    rbps = ps.tile([64, QLEN], f32, tag="ps")
                    nc.tensor.matmul(
                        rbps, r_(ones64[64:65, :]), r_(rinv[64:65, :]), start=True, stop=True
                    )
                    rb = rbp.tile([64, QLEN], f32, tag="rb")
                    nc.vector.tensor_copy(rb, rbps)
                    if po == 0:
                        nc.vector.tensor_mul(ot_sb[f][0:64, :], otp[0:64, :], rb)
                    else:
                        tmp = otmpp.tile([64, QLEN], f32, tag="otmp")
                        nc.vector.tensor_mul(tmp, otp[0:64, :], rb)
                        nc.sync.dma_start(ot_sb[f][64:128, :], tmp)

                # ---------- output projection (transposed to token-major fp16) ----------
                for fo in range(FT):
                    yp = ps.tile([128, QLEN], f32, tag="ps")
                    for c in range(KCH):
                        w = wst2.tile([128, 128], f32, tag="w2")
                        nc.sync.dma_start(
                            w, io["wot"][c * 128 : (c + 1) * 128, fo * 128 : (fo + 1) * 128]
                        )
                        nc.tensor.matmul(
                            yp, r_(w), r_(ot_sb[c]),
                            start=(c == 0), stop=(c == KCH - 1),
                        )
                    ysb = ypool.tile([128, QLEN], f32, tag="y")
                    nc.scalar.activation(ysb, yp, AF.Copy)
                    for t in range(NQT):
                        typ = ps.tile([128, 128], f32, tag="ps")
                        nc.tensor.transpose(
                            typ, ysb[:, t * 128 : (t + 1) * 128], eye128f
                        )
                        nc.scalar.activation(
                            yout[t][:, fo * 128 : (fo + 1) * 128], typ, AF.Copy
                        )
                for t in range(NQT):
                    nc.sync.dma_start(io["yt"][t * 128 : (t + 1) * 128, :], yout[t])

    nc.compile()
    return nc


NGROUPS = int(os.environ.get("K_NGROUPS", "4"))  # pipeline core-groups (4 best)


class _Runner:
    """Cached jitted shard_map dispatch with device-resident parameters.

    Cores are split into NGROUPS groups, each with its own mesh + jitted
    executable, dispatched back-to-back so group g+1's H2D overlaps group
    g's D2H (the axon tunnel is full-duplex)."""

    def __init__(self):
        import jax
        import jax.numpy as jnp
        from jax.sharding import Mesh, PartitionSpec, NamedSharding
        from jax.experimental.shard_map import shard_map
        from concourse import bass2jax, mybir

        self.jax = jax
        bass2jax.install_neuronx_cc_hook()

        nc = _build_nc()
        self.nc = nc

        partition_name = (
            nc.partition_id_tensor.name if nc.partition_id_tensor else None
        )
        in_names, out_names, out_avals = [], [], []
        for alloc in nc.m.functions[0].allocations:
            if not isinstance(alloc, mybir.MemoryLocationSet):
                continue
            name = alloc.memorylocations[0].name
            if alloc.kind == "ExternalInput":
                if name != partition_name:
                    in_names.append(name)
            elif alloc.kind == "ExternalOutput":
                out_names.append(name)
                shape = tuple(alloc.tensor_shape)
                dtype = mybir.dt.np(alloc.dtype)
                out_avals.append(jax.core.ShapedArray(shape, dtype))
        self.in_names = in_names
        self.out_names = out_names
        n_params = len(in_names)
        n_outs = len(out_avals)
        all_in_names = list(in_names) + out_names + (
            [partition_name] if partition_name else []
        )
        donate = tuple(range(n_params, n_params + n_outs))

        def _body(*args):
            operands = list(args)
            if partition_name is not None:
                operands.append(bass2jax.partition_id_tensor())
            outs = bass2jax._bass_exec_p.bind(
                *operands,
                out_avals=tuple(out_avals),
                in_names=tuple(all_in_names),
                out_names=tuple(out_names),
                lowering_input_output_aliases=(),
                sim_require_finite=True,
                sim_require_nnan=True,
                nc=nc,
            )
            return tuple(outs)

        devices = jax.devices()[:NC]
        assert len(devices) == NC, f"need {NC} devices, got {len(jax.devices())}"
        gsz = NC // NGROUPS
        self.gsz = gsz
        self.groups = []
        for g in range(NGROUPS):
            gdev = devices[g * gsz : (g + 1) * gsz]
            mesh = Mesh(np.asarray(gdev), ("core",))
            shard = NamedSharding(mesh, PartitionSpec("core"))
            in_specs = (PartitionSpec("core"),) * (n_params + n_outs)
            out_specs = (PartitionSpec("core"),) * n_outs
            sharded = jax.jit(
                shard_map(
                    _body, mesh=mesh, in_specs=in_specs, out_specs=out_specs,
                    check_rep=False,
                ),
                donate_argnums=donate,
                keep_unused=True,
            )
            zshapes = [(gsz * a.shape[0], *a.shape[1:]) for a in out_avals]
            zdtypes = [a.dtype for a in out_avals]
            mkzeros = jax.jit(
                (lambda zs, zd: lambda: tuple(jnp.zeros(s, d) for s, d in zip(zs, zd)))(
                    zshapes, zdtypes
                ),
                out_shardings=tuple(shard for _ in zshapes),
            )
            self.groups.append(
                {"shard": shard, "sharded": sharded, "mkzeros": mkzeros, "dev": {}}
            )
        # host copies of the currently-uploaded weights (for change detection)
        self.host_w = None

    def put(self, g, name, arr):
        grp = self.groups[g]
        d = self.jax.device_put(arr, grp["shard"])
        grp["dev"][name] = d
        return d

    def ensure_weights(self, Wq, Wk, Wv, Wo, ln_q_w):
        key = (Wq, Wk, Wv, Wo, ln_q_w)
        if self.host_w is not None and all(
            a is b or np.array_equal(a, b) for a, b in zip(self.host_w, key)
        ):
            return
        gsz = self.gsz
        rep = lambda a: np.broadcast_to(a, (gsz, *a.shape)).reshape(
            gsz * a.shape[0], *a.shape[1:]
        )
        wts = {nm: np.ascontiguousarray(w.T) for nm, w in
               (("wqt", Wq), ("wkt", Wk), ("wvt", Wv), ("wot", Wo))}
        eq2 = _build_eq(ln_q_w)
        m_first, m_mid = _build_masks()
        for g in range(NGROUPS):
            for nm, wt in wts.items():
                self.put(g, nm, rep(wt))
            self.put(g, "eq2", rep(eq2))
            if "eye2" not in self.groups[g]["dev"]:
                self.put(g, "eye2", rep(np.eye(2, dtype=np.float32)))
                self.put(g, "eye128f", rep(np.eye(128, dtype=np.float32)))
                self.put(
                    g,
                    "maskt",
                    np.concatenate(
                        [m_first if c % 4 == 0 else m_mid
                         for c in range(g * gsz, (g + 1) * gsz)],
                        axis=0,
                    ),
                )
        self.host_w = tuple(np.array(a, copy=True) for a in key)

    def run(self, xin_builder):
        """xin_builder(g) -> [gsz*KVLEN, H] fp16.

        Returns the per-group output device arrays ([gsz*QLEN, H] fp16 each)
        with D2H copies already in flight."""
        outs = []
        for g, grp in enumerate(self.groups):
            dev_x = self.jax.device_put(xin_builder(g), grp["shard"])
            zeros = grp["mkzeros"]()
            args = [dev_x if nm == "xin" else grp["dev"][nm] for nm in self.in_names]
            out_arrs = grp["sharded"](*args, *zeros)
            outs.append(out_arrs[0])
        for o in outs:
            o.copy_to_host_async()
        return outs


def _get_runner():
    if "runner" not in _CACHE:
        _CACHE["runner"] = _Runner()
    return _CACHE["runner"]


def _build_masks():
    # maskt[j, p, q]: 0 if key (local kv index j*128+p) is visible to query
    # (local index q), else NEG. Window condition is offset-invariant:
    # 0 <= q + 256 - (j*128 + p) <= 256. Chunk-0 cores additionally blank
    # keys whose global position would be negative (the zero padding).
    j = np.arange(NJ)[:, None, None]
    p = np.arange(128)[None, :, None]
    q = np.arange(QLEN)[None, None, :]
    kv = j * 128 + p
    d = q + PAD - kv
    valid = (d >= 0) & (d <= WIN)
    m_mid = np.where(valid, 0.0, NEG).astype(np.float32)
    m_first = np.where(valid & (kv >= PAD), 0.0, NEG).astype(np.float32)
    return m_first.reshape(NJ * 128, QLEN), m_mid.reshape(NJ * 128, QLEN)


def _build_eq(ln_q_w):
    e = np.zeros((2, 128), np.float32)
    p = np.arange(128)
    e[p // 64, p] = ln_q_w[p % 64]
    return e


def _numpy_ref(x, Wq, bq, Wk, bk, Wv, bv, Wo, bo, ln_q_w, ln_q_b, ln_k_w, ln_k_b):
    # General-case fallback (not used for the spec'd inputs).
    def ln(t, g, b):
        m = t.mean(-1, keepdims=True)
        v = ((t - m) ** 2).mean(-1, keepdims=True)
        return (t - m) / np.sqrt(v + EPS) * g + b

    b_, s_ = x.shape[:2]
    q = (x @ Wq.T + bq).reshape(b_, s_, NH, HD)
    k = (x @ Wk.T + bk).reshape(b_, s_, NH, HD)
    v = (x @ Wv.T + bv).reshape(b_, s_, NH, HD)
    q = ln(q, ln_q_w, ln_q_b)
    k = ln(k, ln_k_w, ln_k_b)
    out = np.empty((b_, s_, NH * HD), np.float32)
    i = np.arange(s_)[:, None]
    jj = np.arange(s_)[None, :]
    mask = (jj <= i) & (i - jj <= WIN)
    for bi in range(b_):
        sc = np.einsum("qhd,khd->hqk", q[bi], k[bi]) / np.sqrt(HD)
        sc = np.where(mask[None], sc, -np.inf)
        sc -= sc.max(-1, keepdims=True)
        p = np.exp(sc)
        p /= p.sum(-1, keepdims=True)
        out[bi] = np.einsum("hqk,khd->qhd", p, v[bi]).reshape(s_, NH * HD)
    return out @ Wo.T + bo


def kernel(**inputs):
    x = np.asarray(inputs["x"], np.float32)
    Wq = np.asarray(inputs["Wq"], np.float32)
    Wk = np.asarray(inputs["Wk"], np.float32)
    Wv = np.asarray(inputs["Wv"], np.float32)
    Wo = np.asarray(inputs["Wo"], np.float32)
    ln_q_w = np.asarray(inputs["ln_q_w"], np.float32)
    zeros_ok = all(
        not np.any(np.asarray(inputs[nm], np.float32))
        for nm in ("bq", "bk", "bv", "bo", "ln_q_b", "ln_k_b")
    )
    lnk_ok = np.allclose(np.asarray(inputs["ln_k_w"], np.float32), 1.0)
    if not (zeros_ok and lnk_ok):
        return _numpy_ref(**{k: np.asarray(v, np.float32) for k, v in inputs.items()})

    os.environ["BASS_NEVER_TRACE"] = "1"
    r = _get_runner()
    r.ensure_weights(Wq, Wk, Wv, Wo, ln_q_w)

    # xin[c]: [KVLEN, H] fp16 token-major; rows 0:PAD halo, PAD: main chunk
    gsz = r.gsz

    def xin_builder(g):
        xg = np.empty((gsz, KVLEN, H), np.float16)
        for i, c in enumerate(range(g * gsz, (g + 1) * gsz)):
            b, ch = c // 4, c % 4
            qs = ch * QLEN
            xg[i, PAD:] = x[b, qs : qs + QLEN]
            if ch > 0:
                xg[i, :PAD] = x[b, qs - PAD : qs]
            else:
                xg[i, :PAD] = 0.0
        return xg.reshape(gsz * KVLEN, H)

    outs = r.run(xin_builder)

    # scatter each group's output as soon as its D2H lands (later groups
    # keep streaming while we upcast earlier ones)
    out = np.empty((B, S, H), np.float32)
    for g, o in enumerate(outs):
        yg = np.asarray(o).reshape(gsz, QLEN, H)
        for i, c in enumerate(range(g * gsz, (g + 1) * gsz)):
            b, ch = c // 4, c % 4
            out[b, ch * QLEN : (ch + 1) * QLEN, :] = yg[i]
    return out
